# revision 1
# baseline (speedup 1.0000x reference)
"""GCNII backbone Bass/Trainium2 kernel — 8-core SPMD.

Sharding: nodes row-partitioned across 8 cores (12500/core, padded to 12544).
Edges live on the core that owns their *destination* node.  Host-side graph
preprocessing (index-only work) builds, per core, a destination-sorted padded
edge stream; the device does everything float:

  f0 = relu(x @ lin1_w.T + b)           PE matmul (feat-major), per node tile
  per layer:
    gather f[src] rows from an all-gathered HBM buffer   (gpsimd.dma_gather)
    segment-sum as matmul:  H[f,d] += G_chunk[e,f].T @ B_chunk[e,d]
        where B[e,d] = (col_rel[e]==d) * enorm[e] is built on DVE with a
        single dual-op tensor_scalar (is_equal then mult)
    P = M1.T @ H + M2.T @ x0            (M1=(1-b)I+b*W1, M2=a((1-b)I+b*W2))
    graph-LayerNorm stats: per-tile row-sum/row-sumsq -> 8-core AllReduce
    f_next = relu((P - m)*inv*nw + nb)  one ACT op (per-partition scale/bias)
    PE-transpose back to node-major, AllGather the slice for the next layer
"""

import os
import sys

for _p in ("/opt/trn_rl_repo",):
    if _p not in sys.path:
        sys.path.insert(0, _p)

import math

import ml_dtypes
import numpy as np

import concourse.bacc as bacc
import concourse.bass as bass
import concourse.tile as tile
from concourse import mybir
from concourse.bass_utils import run_bass_kernel_spmd

F32 = mybir.dt.float32
BF16 = mybir.dt.bfloat16
I16 = mybir.dt.int16
AX = mybir.AxisListType
AL = mybir.AluOpType
AF = mybir.ActivationFunctionType

NCORES = 8
D = 128
DIN = 256
L = 4
ALPHA = 0.5
THETA = 1.0
EPS = 1e-5


def full_cfg(N=100000):
    slice_ = N // NCORES
    pad = ((slice_ + 127) // 128) * 128
    nf = NCORES * pad
    wsize = 32768
    bs = -(-(nf - wsize) // 3)  # ceil
    assert bs <= wsize
    wb = [0, bs, 2 * bs, nf - wsize]
    return dict(N=N, SLICE=slice_, PAD=pad, NT=pad // 128, NF=nf,
                WSIZE=wsize, BS=bs, WB=wb, GSZ=3)


def small_cfg():
    # scaled-down config for fast validation runs
    N = 8000
    slice_ = 1000
    pad = 1024
    nf = NCORES * pad
    wsize = 3072
    bs = -(-(nf - wsize) // 3)
    wb = [0, bs, 2 * bs, nf - wsize]
    return dict(N=N, SLICE=slice_, PAD=pad, NT=pad // 128, NF=nf,
                WSIZE=wsize, BS=bs, WB=wb, GSZ=3)


# ---------------------------------------------------------------- host prep
def preprocess(x, edge_index, lin1_w, lin1_b, w1, w2, norm_w, norm_b, cfg):
    N, SLICE, PAD, NT = cfg["N"], cfg["SLICE"], cfg["PAD"], cfg["NT"]
    BS, WB, WSIZE, GSZ = cfg["BS"], cfg["WB"], cfg["WSIZE"], cfg["GSZ"]

    src = np.asarray(edge_index[0], dtype=np.int64)
    dst = np.asarray(edge_index[1], dtype=np.int64)
    sl = np.arange(N, dtype=np.int64)
    srcA = np.concatenate([src, sl])
    dstA = np.concatenate([dst, sl])

    deg = np.bincount(dstA, minlength=N).astype(np.float64)
    dis = 1.0 / np.sqrt(deg)
    en = ((1.0 - ALPHA) * dis[srcA] * dis[dstA]).astype(np.float32)

    addr = (srcA // SLICE) * PAD + (srcA % SLICE)
    core = dstA // SLICE
    lt = (dstA % SLICE) // 128
    colr = ((dstA % SLICE) % 128).astype(np.float32)
    w = np.minimum(addr // BS, 3)
    idx16 = (addr - np.asarray(WB, dtype=np.int64)[w]).astype(np.int64)
    assert idx16.min() >= 0 and idx16.max() < WSIZE

    ngroups = -(-NT // GSZ)
    groups = [list(range(g * GSZ, min((g + 1) * GSZ, NT))) for g in range(ngroups)]

    # per-(core,tile,window) counts -> shared static capacities (mult of 128)
    blk = (core * NT + lt) * 4 + w
    cnt = np.bincount(blk, minlength=NCORES * NT * 4).reshape(NCORES, NT, 4)
    cap = (np.ceil(cnt.max(axis=0) / 128).astype(np.int64)) * 128  # [NT,4]

    # stream block order: for g: for w: for t in group
    border = [(t, wi) for g in groups for wi in range(4) for t in g]
    blk_of = {tw: i for i, tw in enumerate(border)}
    blk_len = np.array([cap[t, wi] for (t, wi) in border], dtype=np.int64)
    blk_start_arr = np.concatenate([[0], np.cumsum(blk_len)])
    S_total = int(blk_start_arr[-1])
    NCH = S_total // 128
    blk_start = {tw: int(blk_start_arr[i]) for i, tw in enumerate(border)}

    call_start = [[0] * 4 for _ in range(ngroups)]
    call_len = [[0] * 4 for _ in range(ngroups)]
    for gi, g in enumerate(groups):
        for wi in range(4):
            call_start[gi][wi] = blk_start[(g[0], wi)]
            call_len[gi][wi] = int(sum(cap[t, wi] for t in g))

    sched = dict(groups=groups, cap=cap, blk_start=blk_start,
                 call_start=call_start, call_len=call_len,
                 S=S_total, NCH=NCH)

    # per-core streams
    per_core = []
    bidx_all = np.array([blk_of[(int(t), int(wi))] for t, wi in zip(lt, w)],
                        dtype=np.int64)
    for c in range(NCORES):
        m = core == c
        bi = bidx_all[m]
        order = np.argsort(bi, kind="stable")
        bi_s = bi[order]
        # rank within block
        cnts = np.bincount(bi_s, minlength=len(border))
        starts_sorted = np.concatenate([[0], np.cumsum(cnts)])[:-1]
        rank = np.arange(len(bi_s)) - starts_sorted[bi_s]
        pos = blk_start_arr[bi_s] + rank

        idx_s = np.zeros(S_total, np.int64)
        col_s = np.zeros(S_total, np.float32)
        en_s = np.zeros(S_total, np.float32)
        idx_s[pos] = idx16[m][order]
        col_s[pos] = colr[m][order]
        en_s[pos] = en[m][order]

        # pack idxs: per gather call, wrap 16 partitions then replicate x8
        idxp = np.zeros((16, S_total // 16), np.int16)
        for gi in range(ngroups):
            for wi in range(4):
                a, ln = call_start[gi][wi], call_len[gi][wi]
                if ln == 0:
                    continue
                seg = idx_s[a:a + ln].astype(np.int16)
                idxp[:, a // 16:(a + ln) // 16] = seg.reshape(ln // 16, 16).T
        idxp = np.tile(idxp, (NCORES, 1))

        # host-built segment matrix B, streamed from HBM on device:
        # B[p, c, d] = enorm of edge (c*128+p) if its col_rel == d else 0
        Bm = np.zeros((NCH, 128, 128), np.float32)
        Bm[np.arange(S_total) // 128, np.arange(S_total) % 128,
           col_s.astype(np.int64)] = en_s
        Bm = np.ascontiguousarray(Bm.transpose(1, 0, 2)).astype(
            ml_dtypes.bfloat16)

        # x slice, transposed+packed on host: xT[j,k,d] = x[row d, 128j+k]
        xs = np.zeros((PAD, DIN), np.float32)
        xs[:SLICE] = np.asarray(x[c * SLICE:(c + 1) * SLICE], np.float32)
        xT = np.ascontiguousarray(
            xs.T.reshape(2, 128, PAD)).astype(ml_dtypes.bfloat16)

        per_core.append(dict(idx=idxp, bmat=Bm, xT=xT))

    # weights
    lw = np.asarray(lin1_w, np.float32)          # [128, 256]
    lin1wT = np.ascontiguousarray(lw.T.reshape(2, 128, 128)).astype(
        ml_dtypes.bfloat16)
    m1 = np.zeros((L, 128, 128), np.float32)
    m2 = np.zeros((L, 128, 128), np.float32)
    eye = np.eye(128, dtype=np.float32)
    for li in range(L):
        beta = float(np.log(THETA / (li + 1) + 1.0))
        m1[li] = (1.0 - beta) * eye + beta * np.asarray(w1[li], np.float32)
        m2[li] = ALPHA * ((1.0 - beta) * eye + beta * np.asarray(w2[li], np.float32))
    consts = dict(
        lin1wT=lin1wT,
        lin1b=np.asarray(lin1_b, np.float32).reshape(128, 1),
        m1=m1.astype(ml_dtypes.bfloat16), m2=m2.astype(ml_dtypes.bfloat16),
        nw=np.asarray(norm_w, np.float32).reshape(128, 1),
        nb=np.asarray(norm_b, np.float32).reshape(128, 1),
        identb=np.eye(128, dtype=np.float32).astype(ml_dtypes.bfloat16),
        identf=np.eye(128, dtype=np.float32),
    )
    return sched, per_core, consts


# ---------------------------------------------------------------- device IR
def build(cfg, sched, debug=None):
    debug = debug or {}
    n_layers = debug.get("n_layers", L)
    no_ar = debug.get("no_ar", False)
    no_gather = debug.get("no_gather", False)
    stop_f0 = debug.get("stop_f0", False)
    N, PAD, NT, NF = cfg["N"], cfg["PAD"], cfg["NT"], cfg["NF"]
    WSIZE, WB, GSZ = cfg["WSIZE"], cfg["WB"], cfg["GSZ"]
    groups, cap = sched["groups"], sched["cap"]
    blk_start, call_start, call_len = (sched["blk_start"], sched["call_start"],
                                      sched["call_len"])
    S, NCH = sched["S"], sched["NCH"]
    inv_nd = 1.0 / (float(N) * float(D))
    tailz = PAD - cfg["SLICE"]  # zero this many trailing dest cols of last tile

    nc = bacc.Bacc("TRN2", target_bir_lowering=False, debug=False,
                   enable_asserts=False, num_devices=NCORES,
                   num_swdge_queues=4)

    t_xT = nc.dram_tensor("xT", [2, 128, PAD], BF16, kind="ExternalInput")
    t_idx = nc.dram_tensor("idx", [128, S // 16], I16, kind="ExternalInput")
    t_b = nc.dram_tensor("bmat", [128, NCH, 128], BF16, kind="ExternalInput")
    t_l1w = nc.dram_tensor("lin1wT", [2, 128, 128], BF16, kind="ExternalInput")
    t_l1b = nc.dram_tensor("lin1b", [128, 1], F32, kind="ExternalInput")
    t_m1 = nc.dram_tensor("m1", [L, 128, 128], BF16, kind="ExternalInput")
    t_m2 = nc.dram_tensor("m2", [L, 128, 128], BF16, kind="ExternalInput")
    t_nw = nc.dram_tensor("nw", [128, 1], F32, kind="ExternalInput")
    t_nb = nc.dram_tensor("nb", [128, 1], F32, kind="ExternalInput")
    t_idb = nc.dram_tensor("identb", [128, 128], BF16, kind="ExternalInput")
    t_idf = nc.dram_tensor("identf", [128, 128], F32, kind="ExternalInput")
    t_y = nc.dram_tensor("y", [PAD, 128], F32, kind="ExternalOutput")

    rg = [list(range(NCORES))]

    with tile.TileContext(nc) as tc:
        with tc.tile_pool(name="res", bufs=1) as res, \
             tc.tile_pool(name="gp", bufs=2) as gp, \
             tc.tile_pool(name="bp", bufs=2) as bp, \
             tc.tile_pool(name="hp", bufs=3) as hp, \
             tc.tile_pool(name="scr", bufs=2) as scrp, \
             tc.tile_pool(name="xt", bufs=3) as xtp, \
             tc.tile_pool(name="fn", bufs=2) as fnp, \
             tc.tile_pool(name="tr", bufs=2) as trp, \
             tc.tile_pool(name="sv", bufs=2) as sv, \
             tc.tile_pool(name="psA", bufs=2, space="PSUM") as psA, \
             tc.tile_pool(name="psB", bufs=2, space="PSUM") as psB, \
             tc.tile_pool(name="psT", bufs=2, space="PSUM") as psT, \
             tc.tile_pool(name="psM", bufs=2, space="PSUM") as psM, \
             tc.tile_pool(name="dram", bufs=1, space="DRAM") as dram:

            f_slice = dram.tile([PAD, 128], BF16)
            f_full = dram.tile([NF, 128], BF16)
            ar_in = dram.tile([1, 8], F32)
            ar_out = dram.tile([1, 8], F32)

            # ---- resident loads
            idx_sb = res.tile([128, S // 16], I16)
            nc.sync.dma_start(idx_sb[:], t_idx[:])
            idb_sb = res.tile([128, 128], BF16)
            nc.sync.dma_start(idb_sb[:], t_idb[:])
            idf_sb = res.tile([128, 128], F32)
            nc.sync.dma_start(idf_sb[:], t_idf[:])
            l1w_sb = res.tile([128, 2, 128], BF16)
            nc.sync.dma_start(l1w_sb[:], t_l1w[:].rearrange("j k f -> k j f"))
            l1b_sb = res.tile([128, 1], F32)
            nc.sync.dma_start(l1b_sb[:], t_l1b[:])
            m1_sb = res.tile([128, L, 128], BF16)
            nc.sync.dma_start(m1_sb[:], t_m1[:].rearrange("l g f -> g l f"))
            m2_sb = res.tile([128, L, 128], BF16)
            nc.sync.dma_start(m2_sb[:], t_m2[:].rearrange("l g f -> g l f"))
            nw_sb = res.tile([128, 1], F32)
            nc.sync.dma_start(nw_sb[:], t_nw[:])
            nb_sb = res.tile([128, 1], F32)
            nc.sync.dma_start(nb_sb[:], t_nb[:])

            x0_sb = res.tile([128, NT, 128], BF16)
            out_sb = res.tile([128, NT, 128], BF16)
            acc_s = res.tile([128, NT], F32)
            acc_q = res.tile([128, NT], F32)
            ones_c = res.tile([128, 1], F32)
            nc.vector.memset(ones_c[:], 1.0)
            ones_r = res.tile([1, 128], F32)
            nc.vector.memset(ones_r[:], 1.0)

            # ---------------- phase 0: f0 = relu(lin1(x)), write f slice, AG
            for g in groups:
                t0, gsz = g[0], len(g)
                xt = xtp.tile([128, 2, GSZ * 128], BF16, tag="xt")
                nc.sync.dma_start(
                    xt[:, :, :gsz * 128],
                    t_xT[:, :, t0 * 128:t0 * 128 + gsz * 128].rearrange(
                        "j k d -> k j d"))
                f0_ps = psB.tile([128, GSZ, 128], F32, tag="P")
                nc.tensor.matmul(f0_ps[:, :gsz, :], l1w_sb[:, 0, :],
                                 xt[:, 0, :gsz * 128], start=True, stop=False)
                nc.tensor.matmul(f0_ps[:, :gsz, :], l1w_sb[:, 1, :],
                                 xt[:, 1, :gsz * 128], start=False, stop=True)
                nc.scalar.activation(x0_sb[:, t0:t0 + gsz, :], f0_ps[:, :gsz, :],
                                     AF.Relu, bias=l1b_sb[:], scale=1.0)
                if g is groups[-1] and tailz > 0:
                    nc.vector.memset(x0_sb[:, NT - 1, 128 - tailz:], 0.0)
                tr_ps = psT.tile([128, GSZ, 128], BF16, tag="T")
                for j, t in enumerate(g):
                    nc.tensor.transpose(tr_ps[:, j, :], x0_sb[:, t, :],
                                        idb_sb[:])
                trs = trp.tile([128, GSZ, 128], BF16, tag="trb")
                nc.vector.tensor_copy(trs[:, :gsz, :], tr_ps[:, :gsz, :])
                nc.sync.dma_start(
                    f_slice[t0 * 128:t0 * 128 + gsz * 128, :].rearrange(
                        "(j d) f -> d j f", j=gsz),
                    trs[:, :gsz, :])
            nc.gpsimd.collective_compute(
                "AllGather", AL.bypass, replica_groups=rg,
                ins=[f_slice.opt()], outs=[f_full.opt()])
            if stop_f0:
                nc.gpsimd.dma_start(t_y[:], f_slice[:])  # cast bf16->f32
                n_layers_eff = 0
            else:
                n_layers_eff = n_layers

            # ---------------- layers
            for li in range(n_layers_eff):
                for gi, g in enumerate(groups):
                    c0 = call_start[gi][0] // 128
                    c1 = (call_start[gi][3] + call_len[gi][3]) // 128
                    bts = bp.tile([128, max(c1 - c0, 1), 128], BF16, tag="b")
                    if c1 > c0:
                        nc.sync.dma_start(bts[:, :c1 - c0, :],
                                          t_b[:, c0:c1, :])
                    gts = {}
                    for wi in range(4):
                        ln = call_len[gi][wi]
                        if ln == 0:
                            continue
                        gt = gp.tile([128, max(ln // 128, 1), 128], BF16,
                                     tag=f"G{wi}")
                        a = call_start[gi][wi]
                        if no_gather:
                            nc.vector.memset(gt[:, :ln // 128, :], 0.0)
                        else:
                            nc.gpsimd.dma_gather(
                                gt[:, :ln // 128, :],
                                f_full[WB[wi]:WB[wi] + WSIZE, :],
                                idx_sb[:, a // 16:(a + ln) // 16],
                                ln, ln, 128, single_packet=False,
                                queue_num=wi)
                        gts[wi] = gt
                    for t in g:
                        ncht = int(cap[t, :].sum()) // 128
                        h_sb = hp.tile([128, 128], BF16, tag="h")
                        if ncht:
                            h_ps = psA.tile([128, 128], F32, tag="H")
                            ci = 0
                            for wi in range(4):
                                nck = int(cap[t, wi]) // 128
                                if nck == 0:
                                    continue
                                cl0 = (blk_start[(t, wi)]
                                       - call_start[gi][wi]) // 128
                                cg0 = blk_start[(t, wi)] // 128
                                for k in range(nck):
                                    nc.tensor.matmul(
                                        h_ps[:], gts[wi][:, cl0 + k, :],
                                        bts[:, cg0 + k - c0, :],
                                        start=(ci == 0), stop=(ci == ncht - 1))
                                    ci += 1
                            nc.scalar.activation(h_sb[:], h_ps[:], AF.Copy)
                        else:
                            nc.vector.memset(h_sb[:], 0.0)
                        p_ps = psB.tile([128, 128], F32, tag="P")
                        nc.tensor.matmul(p_ps[:], m1_sb[:, li, :], h_sb[:],
                                         start=True, stop=False)
                        nc.tensor.matmul(p_ps[:], m2_sb[:, li, :],
                                         x0_sb[:, t, :], start=False, stop=True)
                        nc.scalar.activation(
                            out_sb[:, t, :], p_ps[:], AF.Copy,
                            accum_out=acc_s[:, t:t + 1])
                        scr = scrp.tile([128, 128], BF16, tag="scr")
                        nc.scalar.activation(scr[:], p_ps[:], AF.Square,
                                             accum_out=acc_q[:, t:t + 1])

                # ---- global stats -> AllReduce -> scale/bias vectors
                tot = sv.tile([128, 2], F32, tag="tot")
                nc.vector.tensor_reduce(tot[:, 0:1], acc_s[:, :], axis=AX.X,
                                        op=AL.add)
                nc.vector.tensor_reduce(tot[:, 1:2], acc_q[:, :], axis=AX.X,
                                        op=AL.add)
                st_ps = psM.tile([128, 2], F32, tag="M")
                nc.tensor.matmul(st_ps[0:1, :], ones_c[:], tot[:],
                                 start=True, stop=True)
                st8 = sv.tile([1, 8], F32, tag="st8")
                nc.vector.memset(st8[:], 0.0)
                nc.vector.tensor_copy(st8[0:1, 0:2], st_ps[0:1, :])
                nc.sync.dma_start(ar_in[:], st8[:])
                if no_ar:
                    nc.sync.dma_start(ar_out[:], ar_in[:])
                else:
                    nc.gpsimd.collective_compute(
                        "AllReduce", AL.add, replica_groups=rg,
                        ins=[ar_in.opt()], outs=[ar_out.opt()])
                gs = sv.tile([1, 8], F32, tag="gs")
                nc.sync.dma_start(gs[:], ar_out[:])
                ms = sv.tile([1, 4], F32, tag="ms")
                nc.vector.tensor_scalar(ms[0:1, 0:1], gs[0:1, 0:1], inv_nd,
                                        None, op0=AL.mult)          # m
                nc.vector.tensor_scalar(ms[0:1, 1:2], gs[0:1, 1:2], inv_nd,
                                        None, op0=AL.mult)          # E[x^2]
                nc.vector.tensor_mul(ms[0:1, 2:3], ms[0:1, 0:1], ms[0:1, 0:1])
                nc.vector.tensor_sub(ms[0:1, 3:4], ms[0:1, 1:2], ms[0:1, 2:3])
                sq = sv.tile([1, 4], F32, tag="sq")
                nc.scalar.activation(sq[0:1, 0:1], ms[0:1, 3:4], AF.Sqrt)
                nc.vector.tensor_scalar(sq[0:1, 1:2], sq[0:1, 0:1], EPS, None,
                                        op0=AL.add)
                nc.vector.reciprocal(sq[0:1, 2:3], sq[0:1, 1:2])    # inv
                nc.vector.tensor_mul(sq[0:1, 3:4], sq[0:1, 2:3], ms[0:1, 0:1])
                pk = sv.tile([1, 2], F32, tag="pk")
                nc.vector.tensor_copy(pk[0:1, 0:1], sq[0:1, 2:3])
                nc.vector.tensor_copy(pk[0:1, 1:2], sq[0:1, 3:4])
                bc_ps = psM.tile([128, 2], F32, tag="M")
                nc.tensor.matmul(bc_ps[:], ones_r[:], pk[:],
                                 start=True, stop=True)
                bc = sv.tile([128, 2], F32, tag="bc")
                nc.vector.tensor_copy(bc[:], bc_ps[:])
                scv = sv.tile([128, 1], F32, tag="scv")
                nc.vector.tensor_mul(scv[:], bc[:, 0:1], nw_sb[:])
                bv1 = sv.tile([128, 1], F32, tag="bv1")
                nc.vector.tensor_mul(bv1[:], bc[:, 1:2], nw_sb[:])
                bv = sv.tile([128, 1], F32, tag="bv")
                nc.vector.tensor_sub(bv[:], nb_sb[:], bv1[:])

                # ---- normalize + relu + transpose + store
                last = li == L - 1
                for g in groups:
                    t0, gsz = g[0], len(g)
                    if not last:
                        fn = fnp.tile([128, GSZ, 128], BF16, tag="fnb")
                        nc.scalar.activation(fn[:, :gsz, :],
                                             out_sb[:, t0:t0 + gsz, :],
                                             AF.Relu, bias=bv[:], scale=scv[:])
                        tr_ps = psT.tile([128, GSZ, 128], BF16, tag="T")
                        for j in range(gsz):
                            nc.tensor.transpose(tr_ps[:, j, :], fn[:, j, :],
                                                idb_sb[:])
                        trs = trp.tile([128, GSZ, 128], BF16, tag="trb")
                        nc.vector.tensor_copy(trs[:, :gsz, :], tr_ps[:, :gsz, :])
                        nc.sync.dma_start(
                            f_slice[t0 * 128:t0 * 128 + gsz * 128, :].rearrange(
                                "(j d) f -> d j f", j=gsz),
                            trs[:, :gsz, :])
                    else:
                        fn = fnp.tile([128, GSZ, 128], F32, tag="fnf")
                        nc.scalar.activation(fn[:, :gsz, :],
                                             out_sb[:, t0:t0 + gsz, :],
                                             AF.Relu, bias=bv[:], scale=scv[:])
                        tr_ps = psT.tile([128, GSZ, 128], F32, tag="T")
                        for j in range(gsz):
                            nc.tensor.transpose(tr_ps[:, j, :], fn[:, j, :],
                                                idf_sb[:])
                        trs = trp.tile([128, GSZ, 128], F32, tag="trf")
                        nc.vector.tensor_copy(trs[:, :gsz, :], tr_ps[:, :gsz, :])
                        nc.sync.dma_start(
                            t_y[t0 * 128:t0 * 128 + gsz * 128, :].rearrange(
                                "(j d) f -> d j f", j=gsz),
                            trs[:, :gsz, :])
                if not last:
                    nc.gpsimd.collective_compute(
                        "AllGather", AL.bypass, replica_groups=rg,
                        ins=[f_slice.opt()], outs=[f_full.opt()])

    nc.compile()
    return nc


_last_results = None


def run(inputs, cfg, trace=False):
    global _last_results
    sched, per_core, consts = preprocess(
        inputs["x"], inputs["edge_index"], inputs["lin1_w"], inputs["lin1_b"],
        inputs["w1"], inputs["w2"], inputs["norm_w"], inputs["norm_b"], cfg)
    nc = build(cfg, sched)
    in_maps = []
    for c in range(NCORES):
        m = dict(per_core[c])
        m.update(consts)
        in_maps.append(m)
    _last_results = run_bass_kernel_spmd(
        nc, in_maps, core_ids=list(range(NCORES)), trace=trace)
    SLICE = cfg["SLICE"]
    out = np.concatenate(
        [_last_results.results[c]["y"][:SLICE] for c in range(NCORES)], axis=0)
    return out.astype(np.float32)


def kernel(**inputs):
    return run(inputs, full_cfg(inputs["x"].shape[0]))

